# revision 20
# baseline (speedup 1.0000x reference)
"""Multi-head self-attention (RoPE, causal) Bass kernel for 8 trn2 NeuronCores.

Sharding: (batch, head-group) -> core.  B=2 batches x 4 head-groups of 4 heads.
Each core computes Q/K/V projections for its 4 heads (head dim 64), RoPE,
causal flash-style attention in transposed-score layout, and a partial
O-projection (its rows of wo).  Host sums the 4 partials per batch and adds bo.

Layouts (per core):
  hid_t  [H=1024, S=2048]  hidden^T (host-transposed)
  Q^T/K^T as 2 j-tiles [128, S] (2 heads each, head-major rows)
  V as per-(s-tile, head) tiles [128, 65] (col 64 = ones -> fused softmax denom)
  scores^T tiles [k=128, q=512]; exp on ACT (scale=1/8 folded in); no
  max-subtraction (scores are O(5) for this problem's scales, exp is safe)
  attn^T accum in PSUM [65, 512]; row 64 = denominator
  O^T [H, S] partial written to DRAM.
"""
import sys
import numpy as np

sys.path.insert(0, '/opt/trn_rl_repo')

import concourse.bass as bass
import concourse.bacc as bacc
import concourse.mybir as mybir
from concourse import tile
from concourse.bass_utils import run_bass_kernel_spmd

dt = mybir.dt
F32 = dt.float32
F32R = dt.float32r
BF16 = dt.bfloat16
MMDT = BF16              # dtype of all matmul operands (F32R or BF16)
AF = mybir.ActivationFunctionType

B, S, H, NH = 2, 2048, 1024, 16
HD = 64                  # head dim
NHPC = 4                 # heads per core
J = NHPC * HD            # 256 projected cols per core
SB = 512                 # q-block (free dim of score matmuls)
KB = 128                 # k-block (partition dim of score tiles)
ROPE_BASE = 10000.0
NEG = -1e9


# ---------------------------------------------------------------- host tables
def _host_tables(s: int = S):
    inv_freq = 1.0 / (ROPE_BASE ** (np.arange(0, HD, 2, dtype=np.float64) / HD))
    ang = np.arange(s, dtype=np.float64)[:, None] * inv_freq[None, :]   # [s, 32]
    cosT = np.cos(ang).T.astype(np.float32)                             # [32, S]
    sinT = np.sin(ang).T.astype(np.float32)
    cosf = np.tile(cosT, (4, 1))                                        # [128, S]
    sins = np.tile(np.concatenate([-sinT, sinT], axis=0), (2, 1))       # [128, S]

    perm = np.zeros((128, 128), dtype=np.float32)
    for p in range(128):
        sw = (p // HD) * HD + ((p % HD) + HD // 2) % HD
        perm[sw, p] = 1.0                                   # out = perm.T @ x

    idn = np.eye(128, dtype=np.float32)

    # additive causal masks for the 4 partial (diagonal) k-blocks of a q-block
    kk = np.arange(KB)[:, None]
    qq = np.arange(SB)[None, :]
    cmask = np.concatenate(
        [np.where(qq >= j * KB + kk, 0.0, NEG) for j in range(4)], axis=1
    ).astype(np.float32)                                    # [128, 4*SB]
    return cosf, sins, perm, idn, cmask


# ---------------------------------------------------------------- the program
def build_program(mask_mode: str, s: int = S, h: int = H):
    """mask_mode: 'causal' | 'none' | 'dram'"""
    assert mask_mode in ('causal', 'none', 'dram')
    assert s % SB == 0 and h % 128 == 0
    nsb = s // SB            # q-blocks
    nkb = s // KB            # k-blocks
    hch = h // 128           # contraction chunks for projections
    nst = s // 128           # s-tiles for V

    nc = bacc.Bacc("TRN2", target_bir_lowering=False, debug=False, num_devices=8)

    hid_d = nc.dram_tensor("hid", [h, s], MMDT, kind="ExternalInput").ap()
    w_d = {p: nc.dram_tensor(f"w{p}", [h, J], MMDT, kind="ExternalInput").ap()
           for p in "qkv"}
    b_d = {p: nc.dram_tensor(f"b{p}", [J, 1], F32, kind="ExternalInput").ap()
           for p in "qkv"}
    wo_d = nc.dram_tensor("wo", [J, h], MMDT, kind="ExternalInput").ap()
    cosf_d = nc.dram_tensor("cosf", [128, s], F32, kind="ExternalInput").ap()
    sins_d = nc.dram_tensor("sins", [128, s], F32, kind="ExternalInput").ap()
    perm_d = nc.dram_tensor("perm", [128, 128], MMDT, kind="ExternalInput").ap()
    idn_d = nc.dram_tensor("idn", [128, 128], MMDT, kind="ExternalInput").ap()
    cmask_d = nc.dram_tensor("cmask", [128, 4 * SB], F32, kind="ExternalInput").ap()
    ones_d = nc.dram_tensor("ones", [128, HD], MMDT, kind="ExternalInput").ap()
    sel_d = nc.dram_tensor("sel", [2, 128], MMDT, kind="ExternalInput").ap()
    if mask_mode == 'dram':
        biasT_d = nc.dram_tensor("biasT", [s, s], F32, kind="ExternalInput").ap()
    ot_d = nc.dram_tensor("ot", [h, s], F32, kind="ExternalOutput").ap()

    r = lambda ap: ap  # operands already MMDT

    with tile.TileContext(nc) as tc:
        with tc.tile_pool(name="persist", bufs=1) as pp:
            # persistent SBUF
            w_t = {p: [pp.tile([128, J], MMDT, tag=f"w{p}{c}", name=f"w{p}{c}") for c in range(hch)]
                   for p in "qkv"}
            b_t = {p: pp.tile([128, 2], F32, tag=f"b{p}", name=f"bt{p}") for p in "qkv"}
            wo_t = [pp.tile([128, h], MMDT, tag=f"wo{c}", name=f"wot{c}") for c in range(2)]
            perm_t = pp.tile([128, 128], MMDT, tag="perm")
            ones_t = pp.tile([128, HD], MMDT, tag="ones")
            sel_t = [pp.tile([1, 128], MMDT, tag=f"sel{hl}", name=f"selt{hl}")
                     for hl in range(2)]
            idn_t = pp.tile([128, 128], MMDT, tag="idn")
            cmask_t = pp.tile([128, 4 * SB], F32, tag="cmask")
            qt_t = [pp.tile([128, s], MMDT, tag=f"qt{j}", name=f"qt{j}") for j in range(2)]
            kt_t = [pp.tile([128, s], MMDT, tag=f"kt{j}", name=f"kt{j}") for j in range(2)]
            v_t = [[pp.tile([128, HD + 1], MMDT, tag=f"v{st}_{hh}", name=f"v{st}_{hh}")
                    for hh in range(NHPC)] for st in range(nst)]

            for p in "qkv":
                for c in range(hch):
                    nc.sync.dma_start(w_t[p][c][:], w_d[p][c * 128:(c + 1) * 128, :])
                nc.sync.dma_start(b_t[p][:, 0:1], b_d[p][0:128, 0:1])
                nc.sync.dma_start(b_t[p][:, 1:2], b_d[p][128:256, 0:1])
            for c in range(2):
                nc.sync.dma_start(wo_t[c][:], wo_d[c * 128:(c + 1) * 128, :])
            nc.sync.dma_start(perm_t[:], perm_d)
            nc.sync.dma_start(ones_t[:], ones_d)
            for hl in range(2):
                nc.sync.dma_start(sel_t[hl][:], sel_d[hl:hl + 1, :])
            nc.sync.dma_start(idn_t[:], idn_d)
            nc.sync.dma_start(cmask_t[:], cmask_d)
            for st in range(nst):
                for hh in range(NHPC):
                    nc.vector.tensor_copy(v_t[st][hh][:, HD:HD + 1],
                                          ones_t[:, 0:1])

            # ---------------- phase A: projections + RoPE + V transpose ----
            # 1-deep software pipeline: the epilogue (ACT copy / perm-matmul /
            # transposes / RoPE DVE) of tile i is emitted AFTER tile i+1's
            # projection matmuls so the PE never sits behind an ACT/DVE dep.
            with tc.tile_pool(name="pa_sb", bufs=1) as pa, \
                 tc.tile_pool(name="pa_tr", bufs=3) as pt, \
                 tc.tile_pool(name="ps_proj", bufs=3, space="PSUM") as ps_proj, \
                 tc.tile_pool(name="ps_sw", bufs=2, space="PSUM") as ps_sw, \
                 tc.tile_pool(name="ps_tr", bufs=3, space="PSUM") as ps_tr:

                hid_t = [pa.tile([128, s], MMDT, tag=f"hid{c}", name=f"hid{c}") for c in range(hch)]
                cosf_t = pa.tile([128, s], F32, tag="cosf")
                sins_t = pa.tile([128, s], F32, tag="sins")
                for c in range(hch):
                    nc.sync.dma_start(hid_t[c][:], hid_d[c * 128:(c + 1) * 128, :])
                nc.sync.dma_start(cosf_t[:], cosf_d)
                nc.sync.dma_start(sins_t[:], sins_d)

                def proj_mms(p, jt, sb):
                    ps = ps_proj.tile([128, SB], F32, tag="proj", name="ps")
                    for c in range(hch):
                        nc.tensor.matmul(
                            ps[:], r(w_t[p][c][:, jt * 128:(jt + 1) * 128]),
                            r(hid_t[c][:, sb * SB:(sb + 1) * SB]),
                            start=(c == 0), stop=(c == hch - 1))
                    return ps

                def qk_epilogue(p, jt, sb, ps):
                    def run():
                        dst = qt_t if p == "q" else kt_t
                        raw = pt.tile([128, SB], MMDT, tag="raw", name="raw")
                        nc.scalar.activation(raw[:], ps[:], AF.Identity,
                                             bias=b_t[p][:, jt:jt + 1])
                        sw = ps_sw.tile([128, SB], F32, tag="sw", name="sw")
                        nc.tensor.matmul(sw[:], r(perm_t[:]), r(raw[:]),
                                         start=True, stop=True)
                        t1 = pt.tile([128, SB], F32, tag="t1", name="t1")
                        nc.vector.tensor_mul(
                            t1[:], raw[:], cosf_t[:, sb * SB:(sb + 1) * SB])
                        t2 = pt.tile([128, SB], F32, tag="t2", name="t2")
                        nc.vector.tensor_mul(
                            t2[:], sw[:], sins_t[:, sb * SB:(sb + 1) * SB])
                        nc.vector.tensor_add(
                            dst[jt][:, sb * SB:(sb + 1) * SB], t1[:], t2[:])
                    return run

                def v_epilogue(jt, sb, ps):
                    def run():
                        raw = pt.tile([128, SB], MMDT, tag="raw", name="raw")
                        nc.scalar.activation(raw[:], ps[:], AF.Identity,
                                             bias=b_t["v"][:, jt:jt + 1])
                        for i in range(SB // 128):
                            st = sb * (SB // 128) + i
                            tr = ps_tr.tile([128, 128], MMDT, tag="tr", name="tr")
                            nc.tensor.transpose(
                                tr[:], raw[:, i * 128:(i + 1) * 128], idn_t[:])
                            for hl in range(2):
                                nc.vector.tensor_copy(
                                    v_t[st][jt * 2 + hl][:, 0:HD],
                                    tr[:, hl * HD:(hl + 1) * HD])
                    return run

                pend = None
                for p in "qkv":
                    for jt in range(2):
                        for sb in range(nsb):
                            ps = proj_mms(p, jt, sb)
                            if pend is not None:
                                pend()
                            pend = (v_epilogue(jt, sb, ps) if p == "v"
                                    else qk_epilogue(p, jt, sb, ps))
                if pend is not None:
                    pend()

            # ---------------- phase B: attention + O-projection ------------
            # Scores for both heads of a pair land in one 2-bank [128, 2*SB]
            # PSUM tile -> single exp ACT per k-block.  1-deep pipeline: the
            # epilogue (mask add, exp, attn matmuls) of k-block i is emitted
            # after k-block i+1's score matmuls.  attn accumulators are
            # per-pair (2 banks); denominators are normalized via a batched
            # reciprocal + selector-matrix PE broadcast.
            with tc.tile_pool(name="pb_sb", bufs=2) as pb, \
                 tc.tile_pool(name="pb_p", bufs=3) as pbp, \
                 tc.tile_pool(name="pb_o", bufs=3) as pbo, \
                 tc.tile_pool(name="ps_sc", bufs=2, space="PSUM") as ps_sc, \
                 tc.tile_pool(name="ps_at", bufs=1, space="PSUM") as ps_at, \
                 tc.tile_pool(name="ps_o", bufs=2, space="PSUM") as ps_o:

                for qb in range(nsb):
                    qs = qb * SB
                    if mask_mode == 'causal':
                        kbs = [(kb, None) for kb in range(qb * (SB // KB))] + \
                              [(qb * (SB // KB) + jj, jj) for jj in range(SB // KB)]
                    else:
                        kbs = [(kb, None) for kb in range(nkb)]
                    nki = len(kbs)

                    attn_sb = [pb.tile([128, SB], MMDT, tag=f"asb{jt}",
                                       name=f"asb{jt}") for jt in range(2)]

                    for jt in range(2):
                        attn_ps = [ps_at.tile([HD + 1, SB], F32, tag=f"at{hl}",
                                              name=f"at{hl}") for hl in range(2)]

                        def score_mms(kb):
                            sc2 = ps_sc.tile([128, 2 * SB], F32, tag="sc",
                                             name="sc2")
                            for hl in range(2):
                                nc.tensor.matmul(
                                    sc2[:, hl * SB:(hl + 1) * SB],
                                    r(kt_t[jt][hl * HD:(hl + 1) * HD,
                                               kb * KB:(kb + 1) * KB]),
                                    r(qt_t[jt][hl * HD:(hl + 1) * HD, qs:qs + SB]),
                                    start=True, stop=True)
                            return sc2

                        def kb_epilogue(jtl, ki, kb, jj, sc2):
                            def run():
                                if jj is not None:
                                    for hl in range(2):
                                        nc.vector.tensor_add(
                                            sc2[:, hl * SB:(hl + 1) * SB],
                                            sc2[:, hl * SB:(hl + 1) * SB],
                                            cmask_t[:, jj * SB:(jj + 1) * SB])
                                elif mask_mode == 'dram':
                                    bia = pbo.tile([128, SB], F32, tag="bia",
                                                   name="bia")
                                    nc.sync.dma_start(
                                        bia[:],
                                        biasT_d[kb * KB:(kb + 1) * KB, qs:qs + SB])
                                    for hl in range(2):
                                        nc.vector.tensor_add(
                                            sc2[:, hl * SB:(hl + 1) * SB],
                                            sc2[:, hl * SB:(hl + 1) * SB], bia[:])
                                pexp = pbp.tile([128, 2 * SB], MMDT, tag="pexp",
                                                name="pexp")
                                nc.scalar.activation(pexp[:], sc2[:], AF.Exp,
                                                     scale=1.0 / np.sqrt(HD))
                                for hl in range(2):
                                    nc.tensor.matmul(
                                        attn_ps[hl][:],
                                        r(v_t[kb][jtl * 2 + hl][:]),
                                        r(pexp[:, hl * SB:(hl + 1) * SB]),
                                        start=(ki == 0), stop=(ki == nki - 1))
                            return run

                        pend = None
                        for ki, (kb, jj) in enumerate(kbs):
                            sc2 = score_mms(kb)
                            if pend is not None:
                                pend()
                            pend = kb_epilogue(jt, ki, kb, jj, sc2)
                        pend()

                        # normalize: reciprocal of the fused denom rows, then
                        # broadcast across 64 partitions via K=1 selector
                        # matmuls accumulated into one [128, SB] tile
                        rb = ps_o.tile([128, SB], F32, tag="po", name="rb")
                        for hl in range(2):
                            rcp = pbo.tile([1, SB], MMDT, tag=f"rcp{hl}",
                                           name=f"rcp{hl}")
                            with nc.allow_low_precision(reason="softmax recip"):
                                nc.vector.reciprocal(rcp[:],
                                                     attn_ps[hl][HD:HD + 1, :])
                            nc.tensor.matmul(rb[:], sel_t[hl][:], rcp[:],
                                             start=(hl == 0), stop=(hl == 1))
                        recb = pbo.tile([128, SB], F32, tag="recb", name="recb")
                        nc.vector.tensor_copy(recb[:], rb[:])
                        for hl in range(2):
                            nc.vector.tensor_mul(
                                attn_sb[jt][hl * HD:(hl + 1) * HD, :],
                                attn_ps[hl][0:HD, :],
                                recb[hl * HD:(hl + 1) * HD, :])

                    for ot in range(h // 128):
                        po = ps_o.tile([128, SB], F32, tag="po", name="po")
                        for c in range(2):
                            nc.tensor.matmul(
                                po[:], r(wo_t[c][:, ot * 128:(ot + 1) * 128]),
                                r(attn_sb[c][:]), start=(c == 0), stop=(c == 1))
                        osb = pbo.tile([128, SB], F32, tag="osb", name="osb")
                        nc.scalar.copy(osb[:], po[:])
                        nc.sync.dma_start(ot_d[ot * 128:(ot + 1) * 128, qs:qs + SB],
                                          osb[:])
    nc.compile()
    return nc


_PROG_CACHE: dict = {}
TRACE = False            # test-harness knob: capture NTFF trace
TRACE_CORES = None       # e.g. [0] or list(range(8))
LAST_RESULTS = None      # BassKernelResults of the last run (for test.py)


def _get_program(mask_mode: str):
    if mask_mode not in _PROG_CACHE:
        _PROG_CACHE[mask_mode] = build_program(mask_mode)
    return _PROG_CACHE[mask_mode]


# ------------------------------------------------------------------- wrapper
def kernel(hidden_states, attention_mask, wq, bq, wk, bk, wv, bv, wo, bo):
    import ml_dtypes
    mmnp = np.float32 if MMDT == F32R else ml_dtypes.bfloat16
    hidden_states = np.asarray(hidden_states, dtype=np.float32)
    mask = np.asarray(attention_mask).astype(bool).reshape(B, S, S)

    causal = np.tril(np.ones((S, S), dtype=bool))
    if all(np.array_equal(mask[b], causal) for b in range(B)):
        mask_mode = 'causal'
    elif mask.all():
        mask_mode = 'none'
    else:
        mask_mode = 'dram'

    cosf, sins, perm, idn, cmask = _host_tables()
    nc = _get_program(mask_mode)

    in_maps = []
    for c in range(8):
        b, g = divmod(c, 4)
        js = slice(g * J, (g + 1) * J)
        m = {
            "hid": np.ascontiguousarray(hidden_states[b].T).astype(mmnp),
            "wq": np.ascontiguousarray(np.asarray(wq)[:, js]).astype(mmnp),
            "wk": np.ascontiguousarray(np.asarray(wk)[:, js]).astype(mmnp),
            "wv": np.ascontiguousarray(np.asarray(wv)[:, js]).astype(mmnp),
            "bq": np.ascontiguousarray(bq[js]).reshape(J, 1),
            "bk": np.ascontiguousarray(bk[js]).reshape(J, 1),
            "bv": np.ascontiguousarray(bv[js]).reshape(J, 1),
            "wo": np.ascontiguousarray(np.asarray(wo)[js, :]).astype(mmnp),
            "cosf": cosf, "sins": sins,
            "ones": np.ones((128, HD), dtype=np.float32).astype(mmnp),
            "sel": np.repeat(np.eye(2, dtype=np.float32), HD, axis=1).astype(mmnp),
            "perm": perm.astype(mmnp), "idn": idn.astype(mmnp),
            "cmask": cmask,
        }
        if mask_mode == 'dram':
            m["biasT"] = np.ascontiguousarray(
                np.where(mask[b], 0.0, NEG).T.astype(np.float32))
        in_maps.append(m)

    global LAST_RESULTS
    res = run_bass_kernel_spmd(nc, in_maps, core_ids=list(range(8)),
                               trace=TRACE, trace_cores=TRACE_CORES)
    LAST_RESULTS = res
    out = np.zeros((B, S, H), dtype=np.float32)
    for c in range(8):
        b = c // 4
        out[b] += res.results[c]["ot"].T
    out += np.asarray(bo, dtype=np.float32)
    return out
